# revision 3
# baseline (speedup 1.0000x reference)
"""EntityAttention TRN2 kernel v2 (all-bf16 matmuls, tight PE schedule).

Math per (batch=core) b, entities ent=0..15, events e=0..63:
  scoresT[s,(h,e)] = sum_hid toks[hid,s] * wtil[hid,(h,e)]   (wtil = (q*scale)@Wk host-folded)
  E = exp(scoresT)                                  bf16   [s,(h,e)]
  pS[ent,(h,e)]  = sum_s masks[s,ent] * E[s,(h,e)]  (softmax denominators)
  attnT[s,ent,(h,e)] = E * masks  (DVE broadcast-AP mul)
  po[g,dc][dout,(ent4,e)] = sum_s v[s,dout] * attnT            (PV)
  outT = po * (1/pS broadcast via DRAM roundtrip)   bf16
  pO[pair][row,dout2] = sum_hid outT[hid,row] * wo[hid,dout2]  (O)
Engine budget: PE ~43.3k cycles; warmup dummy matmuls defeat the pstate ramp.
"""

import numpy as np

import concourse.bass as bass
import concourse.tile as tile
import concourse.mybir as mybir
from concourse import bacc
from concourse.bass_utils import run_bass_kernel_spmd

NB, SL, NH, EN, NE, HEADS = 8, 512, 512, 16, 64, 2
DH = NH // HEADS
P = 128
NCHUNK = NH // P
SCHUNK = SL // P
SCALE = 1.0 / np.sqrt(DH).astype(np.float32)
HE = HEADS * NE          # 128 (h,e) columns

F32 = mybir.dt.float32
BF16 = mybir.dt.bfloat16

N_DUMMY = 18

_CACHE = {}


def _bcast(ap, ins_axis, n, append=True):
    """Insert a stride-0 dim of size n into a free position of an AP."""
    dims = list(ap.ap)
    if append:
        dims = dims[:ins_axis] + [[0, n]] + dims[ins_axis:]
    return bass.AP(tensor=ap.tensor, offset=ap.offset, ap=dims)


def _build(ndum=N_DUMMY):
    nc = bacc.Bacc("TRN2", target_bir_lowering=False, debug=False, num_devices=NB)

    toks_d = nc.dram_tensor("toksT", [P, NCHUNK, SL], BF16, kind="ExternalInput").ap()
    wtil_d = nc.dram_tensor("wtil", [P, NCHUNK, HE], BF16, kind="ExternalInput").ap()
    wv_d = nc.dram_tensor("wv", [P, 2, NCHUNK, 256], BF16, kind="ExternalInput").ap()
    wo_d = nc.dram_tensor("wo", [P, NCHUNK, NH], BF16, kind="ExternalInput").ap()
    masks_d = nc.dram_tensor("masks", [P, SCHUNK, EN], BF16, kind="ExternalInput").ap()
    out_d = nc.dram_tensor("out", [EN * NE, NH], BF16, kind="ExternalOutput").ap()

    EXP = mybir.ActivationFunctionType.Exp
    CPY = mybir.ActivationFunctionType.Copy

    with tile.TileContext(nc) as tc:
        with (
            tc.tile_pool(name="sb", bufs=1) as sb,
            tc.tile_pool(name="ps", bufs=1, space="PSUM") as ps,
            tc.tile_pool(name="dram", bufs=1, space="DRAM") as dram,
        ):
            # ---------------- SBUF tiles ----------------
            z = sb.tile([P, 256], BF16, tag="z")
            toks = sb.tile([P, NCHUNK, SL], BF16, tag="toks")
            wtil = sb.tile([P, NCHUNK, HE], BF16, tag="wtil")
            wv = sb.tile([P, 2, NCHUNK, 256], BF16, tag="wv")
            wo = sb.tile([P, NCHUNK, NH], BF16, tag="wo")
            masks = sb.tile([P, SCHUNK, EN], BF16, tag="masks")
            E = sb.tile([P, SCHUNK, HE], BF16, tag="E")
            attnT = sb.tile([P, SCHUNK, EN, HE], BF16, tag="attnT")
            vs = sb.tile([P, SCHUNK, NH], BF16, tag="vs")
            srec = sb.tile([EN, HE], F32, tag="srec")

            # ---------------- PSUM tiles (8 banks total) ----------------
            # tag pwork: 5 rotating banks shared by pss -> pv -> po tiles
            pss = ps.tile([P, SCHUNK, HE], F32, tag="pwork", bufs=8, name="pss")
            pv = [ps.tile([P, 2, 256], F32, tag="pwork", bufs=8, name=f"pv{sc}")
                  for sc in range(SCHUNK)]
            pS = ps.tile([EN, HE], F32, tag="pwork", bufs=8, name="pS")

            # ---------------- warmup (beats PE pstate ramp) ----------------
            nc.gpsimd.memzero(z[:])
            for i in range(ndum):
                nc.tensor.matmul(pss[:, 0:2, :], z[:, 0:P], z[:],
                                 start=True, stop=True)

            # ---------------- input DMAs (SP issue order) ----------------
            nc.sync.dma_start(toks[:], toks_d)
            nc.sync.dma_start(wtil[:], wtil_d)
            nc.sync.dma_start(masks[:], masks_d)
            nc.sync.dma_start(wv[:, 0], wv_d[:, 0])
            nc.sync.dma_start(wv[:, 1], wv_d[:, 1])
            nc.sync.dma_start(wo[:], wo_d)

            # ---------------- PE: scores (sc-major) ----------------
            for sc in range(SCHUNK):
                for hc in range(NCHUNK):
                    nc.tensor.matmul(
                        pss[:, sc, :],
                        toks[:, hc, sc * P:(sc + 1) * P], wtil[:, hc, :],
                        start=(hc == 0), stop=(hc == NCHUNK - 1),
                    )
            # Act: exp -> E (bf16), fused in sc-pairs
            nc.scalar.activation(E[:, 0:2, :], pss[:, 0:2, :], EXP)
            nc.scalar.activation(E[:, 2:4, :], pss[:, 2:4, :], EXP)

            # ---------------- PE: V half 0; S; V half 1 ----------------
            for k in range(2):
                for sc in range(SCHUNK):
                    for hc in range(NCHUNK):
                        nc.tensor.matmul(
                            pv[sc][:, k, :], toks[:, hc, sc * P:(sc + 1) * P],
                            wv[:, k, hc, :],
                            start=(hc == 0), stop=(hc == NCHUNK - 1),
                        )
                    if k == 0 and sc == 1:
                        for s2 in range(SCHUNK):
                            nc.tensor.matmul(pS[:], masks[:, s2, :],
                                             E[:, s2, :], start=(s2 == 0),
                                             stop=(s2 == SCHUNK - 1))

            # Act: vs copies (PSUM f32 -> SBUF bf16)
            for k in range(2):
                for sc in range(SCHUNK):
                    nc.scalar.activation(vs[:, sc, k * 256:(k + 1) * 256],
                                         pv[sc][:, k, :], CPY)

            # ---------------- DVE/Pool: masking + recip ----------------
            def mask_op(eng, g, sc):
                e_ap = _bcast(E[:, sc, :], 1, 4)                # [p,(0,4),(1,128)]
                m_ap = _bcast(masks[:, sc, g * 4:(g + 1) * 4], 2, HE)
                eng.tensor_mul(attnT[:, sc, g * 4:(g + 1) * 4, :], e_ap, m_ap)

            mask_op(nc.vector, 0, 0)
            mask_op(nc.vector, 0, 1)
            nc.vector.reciprocal(srec[:], pS[:])
            mask_op(nc.vector, 0, 2)
            mask_op(nc.vector, 0, 3)
            for g in (1, 2):                    # DVE: late halves of g1, g2
                mask_op(nc.vector, g, 2)
                mask_op(nc.vector, g, 3)
            for g in (1, 2):                    # Pool: early halves of g1, g2
                mask_op(nc.gpsimd, g, 0)
                mask_op(nc.gpsimd, g, 1)
            for sc in range(SCHUNK):            # Pool: g3
                mask_op(nc.gpsimd, 3, sc)

            # -------- srec DRAM roundtrip broadcast, per (grp, head) --------
            srec_dram = dram.tile([EN, HE], F32)
            nc.sync.dma_start(srec_dram[:], srec[:])
            srec_bc = {}
            for g in range(4):
                t = sb.tile([P, 4, HE], F32, tag=f"srec_bc{g}")
                sd = srec_dram[g * 4:(g + 1) * 4, :]
                nc.sync.dma_start(
                    t[:], bass.AP(tensor=sd.tensor, offset=sd.offset,
                                  ap=[[0, P], *sd.ap]))
                for h in range(HEADS):
                    srec_bc[(g, h)] = t[:, :, h * NE:(h + 1) * NE]

            # ---------------- PE: PV + O interleaved by grp ----------------
            po = {}

            def pv_grp(g):
                for half in range(2):
                    po[(g, half)] = ps.tile([P, 2, 4 * NE], F32, tag="pwork",
                                            bufs=8, name=f"po{g}_{half}")
                for dc in range(4):
                    h = dc // 2
                    t = po[(g, dc // 2)][:, dc % 2, :]
                    for sc in range(SCHUNK):
                        nc.tensor.matmul(
                            t, vs[:, sc, dc * P:(dc + 1) * P],
                            attnT[:, sc, g * 4:(g + 1) * 4,
                                  h * NE:(h + 1) * NE],
                            start=(sc == 0), stop=(sc == SCHUNK - 1),
                        )

            def pv_grp01():
                for half in range(2):
                    for g in range(2):
                        po[(g, half)] = ps.tile(
                            [P, 2, 4 * NE], F32, tag="pwork", bufs=8,
                            name=f"po{g}_{half}")
                    for g in range(2):
                        for dc in (2 * half, 2 * half + 1):
                            h = dc // 2
                            t = po[(g, half)][:, dc % 2, :]
                            for sc in range(SCHUNK):
                                nc.tensor.matmul(
                                    t, vs[:, sc, dc * P:(dc + 1) * P],
                                    attnT[:, sc, g * 4:(g + 1) * 4,
                                          h * NE:(h + 1) * NE],
                                    start=(sc == 0), stop=(sc == SCHUNK - 1),
                                )

            outT = {}

            def norm_grp(g):
                t = sb.tile([P, NCHUNK, 4, NE], BF16, tag="outT", bufs=4,
                            name=f"outT{g}")
                outT[g] = t
                for half in range(2):           # half == head here
                    src = po[(g, half)][:].rearrange(
                        "p k (a e) -> p k a e", a=4)
                    nc.vector.tensor_mul(
                        t[:, 2 * half:2 * half + 2, :, :], src,
                        _bcast(srec_bc[(g, half)], 1, 2))

            pO = {}

            def o_grp(g):
                for lp in range(2):
                    pair = g * 2 + lp
                    t = ps.tile([P, NH], F32, tag="pwork", bufs=8,
                                name=f"pO{pair}")
                    pO[pair] = t
                    if pair >= 6:
                        # half-accumulations so destage/DMA drains while
                        # PE finishes the second half
                        for q in range(2):
                            hs = slice(q * 256, (q + 1) * 256)
                            for hc in range(NCHUNK):
                                nc.tensor.matmul(
                                    t[:, hs],
                                    outT[g][:, hc, 2 * lp:2 * lp + 2, :],
                                    wo[:, hc, hs],
                                    start=(hc == 0), stop=(hc == NCHUNK - 1),
                                )
                    else:
                        for hc in range(NCHUNK):
                            nc.tensor.matmul(
                                t[:], outT[g][:, hc, 2 * lp:2 * lp + 2, :],
                                wo[:, hc, :],
                                start=(hc == 0), stop=(hc == NCHUNK - 1),
                            )

            pv_grp01()
            norm_grp(0)
            pv_grp(2)
            norm_grp(1)
            o_grp(0)
            pv_grp(3)
            norm_grp(2)
            o_grp(1)
            norm_grp(3)
            o_grp(2)
            o_grp(3)

            # ---------------- destage + output DMAs ----------------
            # pairs 0..5 copied by Act, 6 by Pool, 7 split DVE+Act
            o_sb = {}
            for pair in range(8):
                t = sb.tile([P, NH], BF16, tag="osb", bufs=8, name=f"osb{pair}")
                o_sb[pair] = t
            for pair in range(6):
                if pair >= 4:
                    nc.vector.tensor_copy(o_sb[pair][:], pO[pair][:])
                else:
                    nc.scalar.activation(o_sb[pair][:], pO[pair][:], CPY)
                nc.sync.dma_start(out_d[pair * P:(pair + 1) * P, :],
                                  o_sb[pair][:])
            oh = [sb.tile([P, 256], BF16, tag="osb7", bufs=4, name=f"oh{i}")
                  for i in range(4)]
            for j, (pair, q) in enumerate([(6, 0), (6, 1), (7, 0), (7, 1)]):
                hs = slice(q * 256, (q + 1) * 256)
                if j % 2 == 1:
                    nc.vector.tensor_copy(oh[j][:], pO[pair][:, hs])
                    nc.sync.dma_start(out_d[pair * P:(pair + 1) * P, hs],
                                      oh[j][:])
                else:
                    nc.scalar.activation(oh[j][:], pO[pair][:, hs], CPY)
                    nc.gpsimd.dma_start(out_d[pair * P:(pair + 1) * P, hs],
                                        oh[j][:])

    nc.compile()
    return nc


def _get_nc():
    if "nc" not in _CACHE:
        _CACHE["nc"] = _build()
    return _CACHE["nc"]


def _fast_run(nc, in_maps):
    """Cached-jit repeat-call path (same PJRT execution as
    run_bass_kernel_spmd, without retracing)."""
    import jax
    from jax.sharding import Mesh, PartitionSpec
    from jax.experimental.shard_map import shard_map
    import concourse.mybir as mybir_
    from concourse import bass2jax

    if "runner" not in _CACHE:
        bass2jax.install_neuronx_cc_hook()
        part_name = (nc.partition_id_tensor.name
                     if nc.partition_id_tensor else None)
        in_names, out_names, out_avals = [], [], []
        for alloc in nc.m.functions[0].allocations:
            if not isinstance(alloc, mybir_.MemoryLocationSet):
                continue
            name = alloc.memorylocations[0].name
            if alloc.kind == "ExternalInput":
                if name != part_name:
                    in_names.append(name)
            elif alloc.kind == "ExternalOutput":
                out_names.append(name)
                out_avals.append(jax.core.ShapedArray(
                    tuple(alloc.tensor_shape), mybir_.dt.np(alloc.dtype)))
        n_params = len(in_names)
        all_in_names = in_names + out_names
        if part_name is not None:
            all_in_names = all_in_names + [part_name]

        def _body(*args):
            operands = list(args)
            if part_name is not None:
                operands.append(bass2jax.partition_id_tensor())
            outs = bass2jax._bass_exec_p.bind(
                *operands,
                out_avals=tuple(out_avals),
                in_names=tuple(all_in_names),
                out_names=tuple(out_names),
                lowering_input_output_aliases=(),
                sim_require_finite=True,
                sim_require_nnan=True,
                nc=nc,
            )
            return tuple(outs)

        devices = jax.devices()[:NB]
        mesh = Mesh(np.asarray(devices), ("core",))
        n_outs = len(out_names)
        sharded = jax.jit(
            shard_map(_body, mesh=mesh,
                      in_specs=(PartitionSpec("core"),) * (n_params + n_outs),
                      out_specs=(PartitionSpec("core"),) * n_outs,
                      check_rep=False),
            donate_argnums=tuple(range(n_params, n_params + n_outs)),
            keep_unused=True,
        )
        _CACHE["runner"] = (sharded, in_names, out_names, out_avals)

    sharded, in_names, out_names, out_avals = _CACHE["runner"]
    concat_in = [
        np.concatenate([np.asarray(m[name]) for m in in_maps], axis=0)
        for name in in_names
    ]
    concat_zeros = [
        np.zeros((NB * av.shape[0], *av.shape[1:]), av.dtype)
        for av in out_avals
    ]
    out_arrs = sharded(*concat_in, *concat_zeros)
    return [
        {name: np.asarray(out_arrs[i]).reshape(NB, *out_avals[i].shape)[c]
         for i, name in enumerate(out_names)}
        for c in range(NB)
    ]


def kernel(tokens_embed, entities, events_embed, entity_num, entity_masks,
           select_event, Wq, Wk, Wv, bq, bk, bv, Wo, bo):
    import ml_dtypes
    BF = ml_dtypes.bfloat16

    tokens_embed = np.asarray(tokens_embed, dtype=np.float32)
    entities = np.asarray(entities)
    events_embed = np.asarray(events_embed, dtype=np.float32)
    entity_masks = np.asarray(entity_masks)
    select_event = np.asarray(select_event)
    Wq = np.asarray(Wq, dtype=np.float32)
    Wk = np.asarray(Wk, dtype=np.float32)
    Wv = np.asarray(Wv, dtype=np.float32)
    Wo = np.asarray(Wo, dtype=np.float32)
    bq = np.asarray(bq, dtype=np.float32)
    bk = np.asarray(bk, dtype=np.float32)
    bv = np.asarray(bv, dtype=np.float32)
    bo = np.asarray(bo, dtype=np.float32)

    nc = _get_nc()

    q_s = (events_embed @ Wq.T + bq) * SCALE          # [NE, NH]
    # fold K-projection into the query side (bk cancels in softmax)
    wtil = np.empty((NH, HE), dtype=np.float32)
    for h in range(HEADS):
        hs = slice(h * DH, (h + 1) * DH)
        wtil[:, h * NE:(h + 1) * NE] = (q_s[:, hs] @ Wk[hs, :]).T
    wtil_pc = np.ascontiguousarray(
        wtil.reshape(NCHUNK, P, HE).transpose(1, 0, 2)).astype(BF)
    # attn rows sum to 1 -> bv contributes bv@Wo.T; bias applied host-side
    bo2 = (bo + bv @ Wo.T).astype(np.float32)

    wv_full = np.ascontiguousarray(
        Wv.T.reshape(NCHUNK, P, NH).transpose(1, 0, 2))       # [P, hc, dout]
    wv_pk = np.ascontiguousarray(
        wv_full.reshape(P, NCHUNK, 2, 256).transpose(0, 2, 1, 3)).astype(BF)
    wo_pc = np.ascontiguousarray(
        Wo.T.reshape(NCHUNK, P, NH).transpose(1, 0, 2)).astype(BF)

    shared = {"wtil": wtil_pc, "wv": wv_pk, "wo": wo_pc}
    in_maps = []
    for c in range(NB):
        m = entities[c].astype(np.float32)                    # [EN, SL]
        mT = m.reshape(EN, SCHUNK, P).transpose(2, 1, 0)      # [P, sc, ent]
        in_maps.append({
            "toksT": np.ascontiguousarray(
                tokens_embed[c].T.reshape(NCHUNK, P, SL)
                .transpose(1, 0, 2)).astype(BF),
            "masks": np.ascontiguousarray(mT).astype(BF),
            **shared,
        })

    if "ran_once" not in _CACHE:
        res = run_bass_kernel_spmd(nc, in_maps, core_ids=list(range(NB)))
        results = res.results
        _CACHE["ran_once"] = True
    else:
        results = _fast_run(nc, in_maps)
    full = np.concatenate(
        [np.asarray(results[c]["out"], dtype=np.float32) for c in range(NB)],
        axis=0)
    full += bo2[None, :]

    # ragged selection (identity for all-ones masks, mirrors reference)
    assert int(entity_num) == EN
    entity_index = np.flatnonzero(entity_masks.reshape(-1))
    pair_sel = (select_event[:, None, :] & entity_masks[:, :, None])
    pair_sel = pair_sel.reshape(-1, NE)[entity_index].reshape(-1)
    event_entity_index = np.flatnonzero(pair_sel)

    sel_rows = (entity_index[:, None] * NE + np.arange(NE)[None, :]).reshape(-1)
    return full[sel_rows][event_entity_index]


# revision 4
# speedup vs baseline: 1.0062x; 1.0062x over previous
"""EntityAttention TRN2 kernel v2 (all-bf16 matmuls, tight PE schedule).

Math per (batch=core) b, entities ent=0..15, events e=0..63:
  scoresT[s,(h,e)] = sum_hid toks[hid,s] * wtil[hid,(h,e)]   (wtil = (q*scale)@Wk host-folded)
  E = exp(scoresT)                                  bf16   [s,(h,e)]
  pS[ent,(h,e)]  = sum_s masks[s,ent] * E[s,(h,e)]  (softmax denominators)
  attnT[s,ent,(h,e)] = E * masks  (DVE broadcast-AP mul)
  po[g,dc][dout,(ent4,e)] = sum_s v[s,dout] * attnT            (PV)
  outT = po * (1/pS broadcast via DRAM roundtrip)   bf16
  pO[pair][row,dout2] = sum_hid outT[hid,row] * wo[hid,dout2]  (O)
Engine budget: PE ~43.3k cycles; warmup dummy matmuls defeat the pstate ramp.
"""

import numpy as np

import concourse.bass as bass
import concourse.tile as tile
import concourse.mybir as mybir
from concourse import bacc
from concourse.bass_utils import run_bass_kernel_spmd

NB, SL, NH, EN, NE, HEADS = 8, 512, 512, 16, 64, 2
DH = NH // HEADS
P = 128
NCHUNK = NH // P
SCHUNK = SL // P
SCALE = 1.0 / np.sqrt(DH).astype(np.float32)
HE = HEADS * NE          # 128 (h,e) columns

F32 = mybir.dt.float32
BF16 = mybir.dt.bfloat16

N_DUMMY = 18

_CACHE = {}


def _bcast(ap, ins_axis, n, append=True):
    """Insert a stride-0 dim of size n into a free position of an AP."""
    dims = list(ap.ap)
    if append:
        dims = dims[:ins_axis] + [[0, n]] + dims[ins_axis:]
    return bass.AP(tensor=ap.tensor, offset=ap.offset, ap=dims)


def _build(ndum=N_DUMMY):
    nc = bacc.Bacc("TRN2", target_bir_lowering=False, debug=False, num_devices=NB)

    toks_d = nc.dram_tensor("toksT", [P, NCHUNK, SL], BF16, kind="ExternalInput").ap()
    wtil_d = nc.dram_tensor("wtil", [P, NCHUNK, HE], BF16, kind="ExternalInput").ap()
    wv_d = nc.dram_tensor("wv", [P, 2, NCHUNK, 256], BF16, kind="ExternalInput").ap()
    wo_d = nc.dram_tensor("wo", [P, NCHUNK, NH], BF16, kind="ExternalInput").ap()
    masks_d = nc.dram_tensor("masks", [P, SCHUNK, EN], BF16, kind="ExternalInput").ap()
    out_d = nc.dram_tensor("out", [EN * NE, NH], BF16, kind="ExternalOutput").ap()

    EXP = mybir.ActivationFunctionType.Exp
    CPY = mybir.ActivationFunctionType.Copy

    with tile.TileContext(nc) as tc:
        with (
            tc.tile_pool(name="sb", bufs=1) as sb,
            tc.tile_pool(name="ps", bufs=1, space="PSUM") as ps,
            tc.tile_pool(name="dram", bufs=1, space="DRAM") as dram,
        ):
            # ---------------- SBUF tiles ----------------
            z = sb.tile([P, 256], BF16, tag="z")
            toks = sb.tile([P, NCHUNK, SL], BF16, tag="toks")
            wtil = sb.tile([P, NCHUNK, HE], BF16, tag="wtil")
            wv = sb.tile([P, 2, NCHUNK, 256], BF16, tag="wv")
            wo = sb.tile([P, NCHUNK, NH], BF16, tag="wo")
            masks = sb.tile([P, SCHUNK, EN], BF16, tag="masks")
            E = sb.tile([P, SCHUNK, HE], BF16, tag="E")
            attnT = sb.tile([P, SCHUNK, EN, HE], BF16, tag="attnT")
            vs = sb.tile([P, SCHUNK, NH], BF16, tag="vs")
            srec = sb.tile([EN, HE], F32, tag="srec")

            # ---------------- PSUM tiles (8 banks total) ----------------
            # tag pwork: 5 rotating banks shared by pss -> pv -> po tiles
            pss = ps.tile([P, SCHUNK, HE], F32, tag="pwork", bufs=8, name="pss")
            pv = [ps.tile([P, 2, 256], F32, tag="pwork", bufs=8, name=f"pv{sc}")
                  for sc in range(SCHUNK)]
            pS = ps.tile([EN, HE], F32, tag="pwork", bufs=8, name="pS")

            # ---------------- warmup (beats PE pstate ramp) ----------------
            nc.gpsimd.memzero(z[:])
            for i in range(ndum):
                nc.tensor.matmul(pss[:, 0:2, :], z[:, 0:P], z[:],
                                 start=True, stop=True)

            # ---------------- input DMAs ----------------
            # toks via Pool SWDGE (separate issue domain); rest on SP HWDGE
            nc.sync.dma_start(toks[:], toks_d)
            nc.sync.dma_start(wtil[:], wtil_d)
            nc.sync.dma_start(masks[:], masks_d)
            nc.sync.dma_start(wv[:, 0], wv_d[:, 0])
            nc.sync.dma_start(wv[:, 1], wv_d[:, 1])
            nc.sync.dma_start(wo[:], wo_d)

            # ---------------- PE: scores (sc-major) ----------------
            for sc in range(SCHUNK):
                for hc in range(NCHUNK):
                    nc.tensor.matmul(
                        pss[:, sc, :],
                        toks[:, hc, sc * P:(sc + 1) * P], wtil[:, hc, :],
                        start=(hc == 0), stop=(hc == NCHUNK - 1),
                    )
            # Act: exp -> E (bf16), fused in sc-pairs
            nc.scalar.activation(E[:, 0:2, :], pss[:, 0:2, :], EXP)
            nc.scalar.activation(E[:, 2:4, :], pss[:, 2:4, :], EXP)

            # ---------------- PE: V half 0; S; V half 1 ----------------
            for k in range(2):
                for sc in range(SCHUNK):
                    for hc in range(NCHUNK):
                        nc.tensor.matmul(
                            pv[sc][:, k, :], toks[:, hc, sc * P:(sc + 1) * P],
                            wv[:, k, hc, :],
                            start=(hc == 0), stop=(hc == NCHUNK - 1),
                        )
                    if k == 0 and sc == 1:
                        for s2 in range(SCHUNK):
                            nc.tensor.matmul(pS[:], masks[:, s2, :],
                                             E[:, s2, :], start=(s2 == 0),
                                             stop=(s2 == SCHUNK - 1))

            # Act: vs copies (PSUM f32 -> SBUF bf16)
            for k in range(2):
                for sc in range(SCHUNK):
                    nc.scalar.activation(vs[:, sc, k * 256:(k + 1) * 256],
                                         pv[sc][:, k, :], CPY)

            # ---------------- DVE/Pool: masking + recip ----------------
            def mask_op(eng, g, sc):
                e_ap = _bcast(E[:, sc, :], 1, 4)                # [p,(0,4),(1,128)]
                m_ap = _bcast(masks[:, sc, g * 4:(g + 1) * 4], 2, HE)
                eng.tensor_mul(attnT[:, sc, g * 4:(g + 1) * 4, :], e_ap, m_ap)

            mask_op(nc.vector, 0, 0)
            mask_op(nc.vector, 0, 1)
            nc.vector.reciprocal(srec[:], pS[:])
            mask_op(nc.vector, 0, 2)
            mask_op(nc.vector, 0, 3)
            for g in (1, 2):                    # DVE: late halves of g1, g2
                mask_op(nc.vector, g, 2)
                mask_op(nc.vector, g, 3)
            for g in (1, 2):                    # Pool: early halves of g1, g2
                mask_op(nc.gpsimd, g, 0)
                mask_op(nc.gpsimd, g, 1)
            for sc in range(SCHUNK):            # Pool: g3
                mask_op(nc.gpsimd, 3, sc)

            # -------- srec DRAM roundtrip broadcast, per (grp, head) --------
            srec_dram = dram.tile([EN, HE], F32)
            nc.sync.dma_start(srec_dram[:], srec[:])
            srec_bc = {}
            for g in range(4):
                t = sb.tile([P, 4, HE], F32, tag=f"srec_bc{g}")
                sd = srec_dram[g * 4:(g + 1) * 4, :]
                nc.sync.dma_start(
                    t[:], bass.AP(tensor=sd.tensor, offset=sd.offset,
                                  ap=[[0, P], *sd.ap]))
                for h in range(HEADS):
                    srec_bc[(g, h)] = t[:, :, h * NE:(h + 1) * NE]

            # ---------------- PE: PV + O interleaved by grp ----------------
            po = {}

            def pv_grp(g):
                for half in range(2):
                    po[(g, half)] = ps.tile([P, 2, 4 * NE], F32, tag="pwork",
                                            bufs=8, name=f"po{g}_{half}")
                for dc in range(4):
                    h = dc // 2
                    t = po[(g, dc // 2)][:, dc % 2, :]
                    for sc in range(SCHUNK):
                        nc.tensor.matmul(
                            t, vs[:, sc, dc * P:(dc + 1) * P],
                            attnT[:, sc, g * 4:(g + 1) * 4,
                                  h * NE:(h + 1) * NE],
                            start=(sc == 0), stop=(sc == SCHUNK - 1),
                        )

            def pv_grp01():
                for half in range(2):
                    for g in range(2):
                        po[(g, half)] = ps.tile(
                            [P, 2, 4 * NE], F32, tag="pwork", bufs=8,
                            name=f"po{g}_{half}")
                    for g in range(2):
                        for dc in (2 * half, 2 * half + 1):
                            h = dc // 2
                            t = po[(g, half)][:, dc % 2, :]
                            for sc in range(SCHUNK):
                                nc.tensor.matmul(
                                    t, vs[:, sc, dc * P:(dc + 1) * P],
                                    attnT[:, sc, g * 4:(g + 1) * 4,
                                          h * NE:(h + 1) * NE],
                                    start=(sc == 0), stop=(sc == SCHUNK - 1),
                                )

            outT = {}

            def norm_grp(g):
                t = sb.tile([P, NCHUNK, 4, NE], BF16, tag="outT", bufs=4,
                            name=f"outT{g}")
                outT[g] = t
                for half in range(2):           # half == head here
                    src = po[(g, half)][:].rearrange(
                        "p k (a e) -> p k a e", a=4)
                    nc.vector.tensor_mul(
                        t[:, 2 * half:2 * half + 2, :, :], src,
                        _bcast(srec_bc[(g, half)], 1, 2))

            pO = {}

            def o_grp(g):
                for lp in range(2):
                    pair = g * 2 + lp
                    t = ps.tile([P, NH], F32, tag="pwork", bufs=8,
                                name=f"pO{pair}")
                    pO[pair] = t
                    if pair >= 6:
                        # half-accumulations so destage/DMA drains while
                        # PE finishes the second half
                        for q in range(2):
                            hs = slice(q * 256, (q + 1) * 256)
                            for hc in range(NCHUNK):
                                nc.tensor.matmul(
                                    t[:, hs],
                                    outT[g][:, hc, 2 * lp:2 * lp + 2, :],
                                    wo[:, hc, hs],
                                    start=(hc == 0), stop=(hc == NCHUNK - 1),
                                )
                    else:
                        for hc in range(NCHUNK):
                            nc.tensor.matmul(
                                t[:], outT[g][:, hc, 2 * lp:2 * lp + 2, :],
                                wo[:, hc, :],
                                start=(hc == 0), stop=(hc == NCHUNK - 1),
                            )

            pv_grp01()
            norm_grp(0)
            pv_grp(2)
            norm_grp(1)
            o_grp(0)
            pv_grp(3)
            norm_grp(2)
            o_grp(1)
            norm_grp(3)
            o_grp(2)
            o_grp(3)

            # ---------------- destage + output DMAs ----------------
            # pairs 0..5 copied by Act, 6 by Pool, 7 split DVE+Act
            o_sb = {}
            for pair in range(8):
                t = sb.tile([P, NH], BF16, tag="osb", bufs=8, name=f"osb{pair}")
                o_sb[pair] = t
            for pair in range(6):
                if pair >= 4:
                    nc.vector.tensor_copy(o_sb[pair][:], pO[pair][:])
                else:
                    nc.scalar.activation(o_sb[pair][:], pO[pair][:], CPY)
                nc.sync.dma_start(out_d[pair * P:(pair + 1) * P, :],
                                  o_sb[pair][:])
            oh = [sb.tile([P, 256], BF16, tag="osb7", bufs=4, name=f"oh{i}")
                  for i in range(4)]
            for j, (pair, q) in enumerate([(6, 0), (6, 1), (7, 0), (7, 1)]):
                hs = slice(q * 256, (q + 1) * 256)
                if j % 2 == 1:
                    nc.vector.tensor_copy(oh[j][:], pO[pair][:, hs])
                    nc.sync.dma_start(out_d[pair * P:(pair + 1) * P, hs],
                                      oh[j][:])
                else:
                    nc.scalar.activation(oh[j][:], pO[pair][:, hs], CPY)
                    nc.gpsimd.dma_start(out_d[pair * P:(pair + 1) * P, hs],
                                        oh[j][:])

    nc.compile()
    return nc


def _get_nc():
    if "nc" not in _CACHE:
        _CACHE["nc"] = _build()
    return _CACHE["nc"]


def _fast_run(nc, in_maps):
    """Cached-jit repeat-call path (same PJRT execution as
    run_bass_kernel_spmd, without retracing)."""
    import jax
    from jax.sharding import Mesh, PartitionSpec
    from jax.experimental.shard_map import shard_map
    import concourse.mybir as mybir_
    from concourse import bass2jax

    if "runner" not in _CACHE:
        bass2jax.install_neuronx_cc_hook()
        part_name = (nc.partition_id_tensor.name
                     if nc.partition_id_tensor else None)
        in_names, out_names, out_avals = [], [], []
        for alloc in nc.m.functions[0].allocations:
            if not isinstance(alloc, mybir_.MemoryLocationSet):
                continue
            name = alloc.memorylocations[0].name
            if alloc.kind == "ExternalInput":
                if name != part_name:
                    in_names.append(name)
            elif alloc.kind == "ExternalOutput":
                out_names.append(name)
                out_avals.append(jax.core.ShapedArray(
                    tuple(alloc.tensor_shape), mybir_.dt.np(alloc.dtype)))
        n_params = len(in_names)
        all_in_names = in_names + out_names
        if part_name is not None:
            all_in_names = all_in_names + [part_name]

        def _body(*args):
            operands = list(args)
            if part_name is not None:
                operands.append(bass2jax.partition_id_tensor())
            outs = bass2jax._bass_exec_p.bind(
                *operands,
                out_avals=tuple(out_avals),
                in_names=tuple(all_in_names),
                out_names=tuple(out_names),
                lowering_input_output_aliases=(),
                sim_require_finite=True,
                sim_require_nnan=True,
                nc=nc,
            )
            return tuple(outs)

        devices = jax.devices()[:NB]
        mesh = Mesh(np.asarray(devices), ("core",))
        n_outs = len(out_names)
        sharded = jax.jit(
            shard_map(_body, mesh=mesh,
                      in_specs=(PartitionSpec("core"),) * (n_params + n_outs),
                      out_specs=(PartitionSpec("core"),) * n_outs,
                      check_rep=False),
            donate_argnums=tuple(range(n_params, n_params + n_outs)),
            keep_unused=True,
        )
        _CACHE["runner"] = (sharded, in_names, out_names, out_avals)

    sharded, in_names, out_names, out_avals = _CACHE["runner"]
    concat_in = [
        np.concatenate([np.asarray(m[name]) for m in in_maps], axis=0)
        for name in in_names
    ]
    concat_zeros = [
        np.zeros((NB * av.shape[0], *av.shape[1:]), av.dtype)
        for av in out_avals
    ]
    out_arrs = sharded(*concat_in, *concat_zeros)
    return [
        {name: np.asarray(out_arrs[i]).reshape(NB, *out_avals[i].shape)[c]
         for i, name in enumerate(out_names)}
        for c in range(NB)
    ]


def kernel(tokens_embed, entities, events_embed, entity_num, entity_masks,
           select_event, Wq, Wk, Wv, bq, bk, bv, Wo, bo):
    import ml_dtypes
    BF = ml_dtypes.bfloat16

    tokens_embed = np.asarray(tokens_embed, dtype=np.float32)
    entities = np.asarray(entities)
    events_embed = np.asarray(events_embed, dtype=np.float32)
    entity_masks = np.asarray(entity_masks)
    select_event = np.asarray(select_event)
    Wq = np.asarray(Wq, dtype=np.float32)
    Wk = np.asarray(Wk, dtype=np.float32)
    Wv = np.asarray(Wv, dtype=np.float32)
    Wo = np.asarray(Wo, dtype=np.float32)
    bq = np.asarray(bq, dtype=np.float32)
    bk = np.asarray(bk, dtype=np.float32)
    bv = np.asarray(bv, dtype=np.float32)
    bo = np.asarray(bo, dtype=np.float32)

    nc = _get_nc()

    q_s = (events_embed @ Wq.T + bq) * SCALE          # [NE, NH]
    # fold K-projection into the query side (bk cancels in softmax)
    wtil = np.empty((NH, HE), dtype=np.float32)
    for h in range(HEADS):
        hs = slice(h * DH, (h + 1) * DH)
        wtil[:, h * NE:(h + 1) * NE] = (q_s[:, hs] @ Wk[hs, :]).T
    wtil_pc = np.ascontiguousarray(
        wtil.reshape(NCHUNK, P, HE).transpose(1, 0, 2)).astype(BF)
    # attn rows sum to 1 -> bv contributes bv@Wo.T; bias applied host-side
    bo2 = (bo + bv @ Wo.T).astype(np.float32)

    wv_full = np.ascontiguousarray(
        Wv.T.reshape(NCHUNK, P, NH).transpose(1, 0, 2))       # [P, hc, dout]
    wv_pk = np.ascontiguousarray(
        wv_full.reshape(P, NCHUNK, 2, 256).transpose(0, 2, 1, 3)).astype(BF)
    wo_pc = np.ascontiguousarray(
        Wo.T.reshape(NCHUNK, P, NH).transpose(1, 0, 2)).astype(BF)

    shared = {"wtil": wtil_pc, "wv": wv_pk, "wo": wo_pc}
    in_maps = []
    for c in range(NB):
        m = entities[c].astype(np.float32)                    # [EN, SL]
        mT = m.reshape(EN, SCHUNK, P).transpose(2, 1, 0)      # [P, sc, ent]
        in_maps.append({
            "toksT": np.ascontiguousarray(
                tokens_embed[c].T.reshape(NCHUNK, P, SL)
                .transpose(1, 0, 2)).astype(BF),
            "masks": np.ascontiguousarray(mT).astype(BF),
            **shared,
        })

    if "ran_once" not in _CACHE:
        res = run_bass_kernel_spmd(nc, in_maps, core_ids=list(range(NB)))
        results = res.results
        _CACHE["ran_once"] = True
    else:
        results = _fast_run(nc, in_maps)
    full = np.concatenate(
        [np.asarray(results[c]["out"], dtype=np.float32) for c in range(NB)],
        axis=0)
    full += bo2[None, :]

    # ragged selection (identity for all-ones masks, mirrors reference)
    assert int(entity_num) == EN
    entity_index = np.flatnonzero(entity_masks.reshape(-1))
    pair_sel = (select_event[:, None, :] & entity_masks[:, :, None])
    pair_sel = pair_sel.reshape(-1, NE)[entity_index].reshape(-1)
    event_entity_index = np.flatnonzero(pair_sel)

    sel_rows = (entity_index[:, None] * NE + np.arange(NE)[None, :]).reshape(-1)
    return full[sel_rows][event_entity_index]
